# revision 44
# baseline (speedup 1.0000x reference)
"""Trainium2 Bass kernel for nn_CapLayer_90056874263182.

Math note: the reference initializes routing logits b0 = zeros, so the
softmax over the 10 output caps starts uniform; s, v and delta_b are then
identical across caps, so the logits stay equal across caps through every
routing iteration and the softmax stays uniform forever.  The routing loop
therefore collapses exactly to

    v[b, o, :] = squash((1/10) * sum_i pred[b, i, :])   for every o

and  sum_i pred[b,i,:] = sum_{c,i} xr[b,c,i] * W[c//8,:,i] + 144*sum_s Wb[s,:]
where xr[b,c,i] = sum over the 18 spatial positions p with p%8 == i of
x[b,c,p]  (the row-major reshape maps in_dim to p%8).

Kernel per core (64 batches):
  - x streams in through Pool/SWDGE cast-DMAs (f32 -> fp16).  Source runs
    are 1152B channel-pair lines, destination runs 576B fp16 — still full
    descriptor efficiency, at HALF the SBUF-side bytes.  fp16 keeps 10
    mantissa bits (~5e-4 relative), far inside the 2e-2 gate.
  - DVE folds each tile in two steps: a fully-contiguous fp16 add of the
    channel-pair halves (DVE 2x mode), then a strided q(18)-reduce to
    xr[cp, b*8+i] in fp16.
  - PE: one K=1 ones-matmul adds the routing bias row (its own tiny early
    f32 DMA), then 8 accumulating fp16 matmuls (1 PE cycle/row) -> S [64,16]
    f32 in PSUM.
  - squash: one DVE tensor_tensor_reduce gives nsq = |S/10|^2, ACT Sqrt +
    DVE reciprocal give the coefficient, one dual-scalar DVE mult -> vrow;
    plain SP DMA writes v.
"""

import numpy as np

BS = 512          # full batch
NC = 8            # cores
B = BS // NC      # batches per core
# HEAD batches ride the SP/HWDGE ring in f32 (first byte at ~2.0us, a
# fused 36-fold reduce starts the DVE early); the rest stream through
# Pool/SWDGE cast-DMAs in fp16.  Pool desc-gen costs 994 + 43.5*nb per
# tile against a 204.8ns/batch transfer slot, so bulk tiles stay >= 7
# batches; the taper keeps the final reduce tail short.
HEAD = 2
SUBS = [7, 7, 7, 7, 7, 7, 6, 6, 5, 2, 1]
# Fold methods: tiles 0..6 fold via the 2x-mode fp16 add tree with the
# L2..L6 levels amortized over SPANS (tile-aligned); tiles 7..8 get
# per-tile trees; tile 9 (2 batches) a 2-op fold; tile 10 (1 batch) one
# fused 36-fold reduce — the minimum-latency primitive after the final
# DMA byte.
NSPAN_TILES = 7
SPANS = [14, 14, 14, 6]   # batches, = tile pairs (0-1, 2-3, 4-5) and 6
NTREE_TILES = 9           # tiles 7..8 add per-tile tree levels
CH = 256          # channels
HW = 144          # h*w
Q = 18            # spatial positions per mod-8 bucket
I8 = 8            # in_dim (= p % 8 bucket)
D = 16            # out_dim
NO = 10           # num output caps

assert HEAD + sum(SUBS) == B
assert sum(SPANS) == sum(SUBS[:NSPAN_TILES])


def _build_nc():
    from contextlib import ExitStack

    import concourse.bass as bass
    import concourse.mybir as mybir
    import concourse.tile as tile
    from concourse import bacc

    f32 = mybir.dt.float32
    f16 = mybir.dt.float16
    AF = mybir.ActivationFunctionType

    # Bacc (not plain Bass): its finalize() runs the sync legalization
    # (event semaphores / matmul-wait moves) that splits multi-wait
    # instructions the TRN2 ISA can't encode.
    # 64KB SWDGE scratch = 4096-descriptor ring so the cast-DMA descriptor
    # generation runs well ahead of the transfer slots.
    nc = bacc.Bacc(dynamic_dma_scratch_size=65536)
    x = nc.dram_tensor("x", [B, CH, HW], f32, kind="ExternalInput")
    # fp16 weight matrix wq[p, i*16+d] = W[p//4, d, i] (host pre-packed)
    wq = nc.dram_tensor("wq", [128, I8 * D], f16, kind="ExternalInput")
    # f32 bias row [1, 16] = 144 * sum_s Wb[s, :]
    wb = nc.dram_tensor("wb", [1, D], f32, kind="ExternalInput")
    # one row per batch; the 10 identical caps are replicated host-side
    # during the unshard (they are mathematically equal, see module doc)
    v = nc.dram_tensor("v", [B, D], f32, kind="ExternalOutput")

    with tile.TileContext(nc) as tc, ExitStack() as ctx:
        consts = ctx.enter_context(tc.tile_pool(name="consts", bufs=1))
        xpool = ctx.enter_context(tc.tile_pool(name="xin", bufs=len(SUBS)))
        xhpool = ctx.enter_context(tc.tile_pool(name="xh", bufs=len(SUBS)))
        xrpool = ctx.enter_context(tc.tile_pool(name="xr", bufs=1))
        small = ctx.enter_context(tc.tile_pool(name="small", bufs=1))
        psum = ctx.enter_context(tc.tile_pool(name="psum", bufs=1, space="PSUM"))

        # f32 head on the SP/HWDGE ring: first bytes flow at ~2.0us while
        # the Pool desc-gen for the cast stream is still priming.
        xhead = xpool.tile([128, HEAD * 2 * HW], f32, tag="xhead", bufs=1)
        nc.sync.dma_start(
            xhead[:, :],
            x[0:HEAD].rearrange("b (cp cl) p -> cp b (cl p)", cp=128),
        )

        # x cast-stream: tile [cp, b, (cl p)] fp16; src runs are the
        # 1152B channel-pair lines, dst runs their 576B fp16 images.
        from collections import Counter

        size_counts = Counter(SUBS)
        xts = []
        x_insts = []
        off = HEAD
        for nb in SUBS:
            xt = xpool.tile(
                [128, nb * 2 * HW], f16, tag=f"xt{nb}", bufs=size_counts[nb]
            )
            src = x[off : off + nb].rearrange("b (cp cl) p -> cp b (cl p)", cp=128)
            dst = xt[:, :].rearrange("c (b clp) -> c b clp", clp=2 * HW)
            x_insts.append(nc.gpsimd.dma_start(dst, src))
            xts.append(xt)
            off += nb

        # bias row (tiny, f32) rides the SP/HWDGE ring early.  The fp16
        # weights go through Pool/SWDGE with a no-sync (scheduler-order)
        # dep on the last x tile, so their DMA_ENGINES slot lands BEHIND
        # the final x byte instead of mid-stream: the last reduce then
        # hangs off a byte that is 182ns earlier.
        brow = consts.tile([1, D], f32)
        nc.sync.dma_start(brow[:, :], wb[:, :])
        wpk = consts.tile([128, I8 * D], f16)
        wpk_inst = nc.gpsimd.dma_start(wpk[:, :], wq[:, :])
        from concourse.bass import InstructionNameOrderedSet

        _deps = InstructionNameOrderedSet()
        _deps.add(x_insts[-1].ins.name)
        wpk_inst.ins.add_nosync_dependencies_from(_deps)
        ones = consts.tile([1, B], f32)
        nc.vector.memset(ones[:, :], 1.0)
        # DVE warm-up, then an early ACT Sqrt pins the sqrt_and_others
        # activation table (holds Sqrt, Square and Copy).
        scr = consts.tile([1, 1], f32)
        nc.vector.tensor_copy(scr[:, :], ones[0:1, 0:1])
        scr2 = consts.tile([1, 1], f32)
        nc.scalar.activation(scr2[:, :], scr[:, :], AF.Sqrt)

        # DVE fold.  Bulk: an fp16 binary add-tree, every level in the DVE
        # 2x mode (all operands fp16 with packed innermost i):
        #   L1 (per DMA tile):  t1[c,b,q,i]  = xt[c,b,0,q,i] + xt[c,b,1,q,i]
        #   L2..L6:             fold q 18 -> 9 -> 4(+rem) -> 2 -> 1 (+rem)
        # ~140 DVE cycles/batch vs 288 for a flat reduce.  L2..L6 run over
        # SPANS for the bulk (amortizing instruction inits) and per-tile
        # for tiles 7..8; ops are emitted level-major so no op waits on
        # its immediate predecessor (the ~95ns same-engine RAW stall).
        # Tile 9 (2 batches) folds in 2 ops; tile 10 (1 batch) in ONE
        # fused 36-fold reduce hanging off the final DMA byte.
        xr = xrpool.tile([128, B * I8], f16)
        ntb = sum(SUBS[:NTREE_TILES])  # batches folded via trees
        t1 = xhpool.tile(
            [128, (ntb + SUBS[NTREE_TILES]) * HW], f16, tag="t1", bufs=1
        )
        t2 = xhpool.tile([128, ntb * 9 * I8], f16, tag="t2", bufs=1)
        t3 = xhpool.tile([128, ntb * 4 * I8], f16, tag="t3", bufs=1)
        t4 = xhpool.tile([128, ntb * 2 * I8], f16, tag="t4", bufs=1)
        t5 = xhpool.tile([128, ntb * I8], f16, tag="t5", bufs=1)

        def qv(tile, width, b0, nb, q0, q1):
            """[c, b-range, q0:q1, i] view of a (b, width-q, i) laid tile."""
            v = tile[:, b0 * width * I8 : (b0 + nb) * width * I8]
            return v.rearrange("c (b q i) -> c b q i", q=width, i=I8)[
                :, :, q0:q1
            ]

        def emit_l1(t, b0):
            nb = SUBS[t]
            half = xts[t][:, :].rearrange(
                "c (b cl qi) -> c cl b qi", cl=2, qi=HW
            )
            nc.vector.tensor_tensor(
                out=qv(t1, Q, b0, nb, 0, Q),
                in0=half[:, 0].rearrange("c b (q i) -> c b q i", i=I8),
                in1=half[:, 1].rearrange("c b (q i) -> c b q i", i=I8),
                op=mybir.AluOpType.add,
            )

        LEVELS = [
            (t1, Q, t2, 9, 0, 9, 9, 18),      # 18 -> 9
            (t2, 9, t3, 4, 0, 4, 4, 8),       # 9  -> 4 (+rem q=8)
            (t3, 4, t4, 2, 0, 2, 2, 4),       # 4  -> 2
            (t4, 2, t5, 1, 0, 1, 1, 2),       # 2  -> 1
        ]

        def emit_level(lvl, b0, nb):
            if lvl < 4:
                src, sw, dst, dw, a0, a1, q0, q1 = LEVELS[lvl]
                nc.vector.tensor_tensor(
                    out=qv(dst, dw, b0, nb, 0, dw),
                    in0=qv(src, sw, b0, nb, a0, a1),
                    in1=qv(src, sw, b0, nb, q0, q1),
                    op=mybir.AluOpType.add,
                )
            else:  # L6: xr = t5 + leftover q'=8 of t2
                nc.vector.tensor_tensor(
                    out=qv(xr, 1, HEAD + b0, nb, 0, 1),
                    in0=qv(t5, 1, b0, nb, 0, 1),
                    in1=qv(t2, 9, b0, nb, 8, 9),
                    op=mybir.AluOpType.add,
                )

        def emit_fused(src_ap, xr_b0, nb, clq):
            red_in = src_ap.rearrange(
                "c (b clq i) -> c b i clq", clq=clq, i=I8
            )
            nc.vector.tensor_reduce(
                out=xr[:, xr_b0 * I8 : (xr_b0 + nb) * I8],
                in_=red_in,
                axis=mybir.AxisListType.X,
                op=mybir.AluOpType.add,
            )

        tree_off = []  # batch offset (within tree space) per tile
        o = 0
        for t in range(NTREE_TILES):
            tree_off.append(o)
            o += SUBS[t]

        with nc.allow_low_precision("fp16 fold feeds fp16 matmul"):
            # head: fused f32 36-fold -> fp16 xr rows 0..HEAD
            emit_fused(xhead[:, :], 0, HEAD, 2 * Q)
            # L1 per SWDGE tile (tree tiles)
            for t in range(NTREE_TILES):
                emit_l1(t, tree_off[t])
            # L2..L6 level-major: spans first, then per-tile for 7..8
            for lvl in range(5):
                b0 = 0
                for nb in SPANS:
                    emit_level(lvl, b0, nb)
                    b0 += nb
                for t in range(NSPAN_TILES, NTREE_TILES):
                    emit_level(lvl, tree_off[t], SUBS[t])
            # tile 9 (2 batches): cl-add into t1 scratch + one q-reduce
            t9 = NTREE_TILES
            nb9 = SUBS[t9]
            b9 = HEAD + sum(SUBS[:t9])
            emit_l1(t9, ntb)
            xh9 = t1[:, ntb * HW : (ntb + nb9) * HW].rearrange(
                "c (b q i) -> c b i q", q=Q, i=I8
            )
            nc.vector.tensor_reduce(
                out=xr[:, b9 * I8 : (b9 + nb9) * I8],
                in_=xh9,
                axis=mybir.AxisListType.X,
                op=mybir.AluOpType.add,
            )
            # tile 10 (1 batch): single fused 36-fold off the last DMA byte
            t10 = NTREE_TILES + 1
            emit_fused(
                xts[t10][:, :], HEAD + sum(SUBS[:t10]), SUBS[t10], 2 * Q
            )

        # S[b, d] = brow[d] + sum_{p, i} xr[p, b*8+i] * wpk[p, i*16+d]
        # bias via a K=1 ones-matmul (brow arrives ~4us in), then 8
        # accumulating fp16 matmuls at 1 PE cycle/row.
        ps = psum.tile([B, D], f32)
        nc.tensor.matmul(ps[:, :], ones[:, :], brow[:, :], start=True, stop=False)
        xr_v = xr[:, :].rearrange("c (b i) -> c i b", i=I8)
        for i in range(I8):
            nc.tensor.matmul(
                ps[:, :],
                xr_v[:, i, :],
                wpk[:, i * D : (i + 1) * D],
                start=False,
                stop=(i == I8 - 1),
            )

        # squash with m = S/10 folded into the scales:
        #   nsq = |m|^2 = 0.01 * sum_d S^2,  rt = 0.1*sqrt(nsq),
        #   v_row = S * rt / (1 + nsq)
        # (a DVE square would avoid ACT's 187ns accumulator read, but the
        # walrus verifier rejects TensorTensor with two PSUM operands)
        sq = small.tile([B, D], f32)
        nsq = small.tile([B, 1], f32)
        nc.scalar.activation(
            sq[:, :], ps[:, :], AF.Square, scale=0.1, accum_out=nsq[:, :]
        )
        rt = small.tile([B, 1], f32)
        nc.scalar.activation(rt[:, :], nsq[:, :], AF.Sqrt, scale=0.01)
        # den/rec on DVE overlap the ACT Sqrt
        den = small.tile([B, 1], f32)
        nc.vector.tensor_scalar_add(den[:, :], nsq[:, :], 1.0)
        rec = small.tile([B, 1], f32)
        nc.vector.reciprocal(rec[:, :], den[:, :])

        # v_row = (S * rt) * rec in one dual-scalar DVE op
        vrow = small.tile([B, D], f32)
        nc.vector.tensor_scalar(
            vrow[:, :],
            ps[:, :],
            rt[:, :],
            rec[:, :],
            op0=mybir.AluOpType.mult,
            op1=mybir.AluOpType.mult,
        )
        nc.sync.dma_start(v[:, :], vrow[:, :])

    nc.finalize()
    return nc


def _host_inputs(x, W, Wb):
    x = np.ascontiguousarray(np.asarray(x, dtype=np.float32)).reshape(BS, CH, HW)
    W = np.asarray(W, dtype=np.float32)
    Wb = np.asarray(Wb, dtype=np.float32)

    # wq[p, i*16 + d] = W[p//4, d, i]  (channel-pair p covers channels
    # 2p, 2p+1, both in group p//4; the cl-pair sum happens on the DVE)
    wrj = np.empty((I8, 128, D), dtype=np.float32)
    s_of_p = np.arange(128) // 4
    for i in range(I8):
        wrj[i] = W[s_of_p, :, i]
    wq = np.ascontiguousarray(
        wrj.transpose(1, 0, 2).reshape(128, I8 * D).astype(np.float16)
    )

    # brow[d] = 144 * sum_s Wb[s, d]  (the /10 happens in the ACT scale)
    wb = np.ascontiguousarray(HW * Wb.sum(axis=0, dtype=np.float64).astype(np.float32))
    return x, wq, wb.reshape(1, D)


def _run(x, W, Wb, trace=False):
    from concourse.bass_utils import run_bass_kernel_spmd

    xs, wq, wb = _host_inputs(x, W, Wb)
    nc = _build_nc()
    in_maps = [
        {"x": np.ascontiguousarray(xs[k * B : (k + 1) * B]), "wq": wq, "wb": wb}
        for k in range(NC)
    ]
    res = run_bass_kernel_spmd(nc, in_maps, list(range(NC)), trace=trace)
    rows = np.concatenate([res.results[k]["v"] for k in range(NC)], axis=0)
    # unshard: replicate the (identical) caps into the full [BS, NO, D] shape
    out = np.ascontiguousarray(
        np.broadcast_to(rows.reshape(BS, 1, D), (BS, NO, D)), dtype=np.float32
    )
    return out, res


def _numpy_fallback(x, W, Wb, b0):
    """Generic routing on the host — only used if b0 is ever nonzero
    (the spec fills b0 with zeros, which collapses the routing; see top)."""
    x = np.asarray(x, np.float32)
    W = np.asarray(W, np.float32)
    Wb = np.asarray(Wb, np.float32)
    b0 = np.asarray(b0, np.float32)
    u = x.reshape(BS, 32, HW, I8)
    pred = np.einsum("bsni,soi->bsno", u, W) + Wb[None, :, None, :]
    pred = pred.reshape(BS, 32 * HW, D)
    b = np.broadcast_to(b0, (BS,) + b0.shape).copy()
    v = None
    for _ in range(3):
        e = np.exp(b - b.max(axis=1, keepdims=True))
        c = e / e.sum(axis=1, keepdims=True)
        s = np.einsum("boi,bid->bod", c, pred)
        nrm = np.linalg.norm(s, axis=2)
        coeff = (nrm * nrm / (1.0 + nrm * nrm)) / nrm
        v = s * coeff[:, :, None]
        b = b + np.einsum("bid,bod->boi", pred, v)
    return v.astype(np.float32)


def kernel(x, W, Wb, b0=None, **_ignored):
    if b0 is not None and np.any(np.asarray(b0)):
        return _numpy_fallback(x, W, Wb, b0)
    try:
        out, _ = _run(x, W, Wb, trace=False)
    except Exception:
        # one retry: the axon-tunneled device occasionally reports a
        # transient NRT_EXEC_UNIT_UNRECOVERABLE on first touch
        out, _ = _run(x, W, Wb, trace=False)
    return out


def kernel_traced(x, W, Wb, b0=None):
    """Like kernel() but also returns the BassKernelResults (exec_time_ns)."""
    return _run(x, W, Wb, trace=True)


# revision 46
# speedup vs baseline: 1.0093x; 1.0093x over previous
"""Trainium2 Bass kernel for nn_CapLayer_90056874263182.

Math note: the reference initializes routing logits b0 = zeros, so the
softmax over the 10 output caps starts uniform; s, v and delta_b are then
identical across caps, so the logits stay equal across caps through every
routing iteration and the softmax stays uniform forever.  The routing loop
therefore collapses exactly to

    v[b, o, :] = squash((1/10) * sum_i pred[b, i, :])   for every o

and  sum_i pred[b,i,:] = sum_{c,i} xr[b,c,i] * W[c//8,:,i] + 144*sum_s Wb[s,:]
where xr[b,c,i] = sum over the 18 spatial positions p with p%8 == i of
x[b,c,p]  (the row-major reshape maps in_dim to p%8).

Kernel per core (64 batches):
  - x streams in through Pool/SWDGE cast-DMAs (f32 -> fp16).  Source runs
    are 1152B channel-pair lines, destination runs 576B fp16 — still full
    descriptor efficiency, at HALF the SBUF-side bytes.  fp16 keeps 10
    mantissa bits (~5e-4 relative), far inside the 2e-2 gate.
  - DVE folds each tile in two steps: a fully-contiguous fp16 add of the
    channel-pair halves (DVE 2x mode), then a strided q(18)-reduce to
    xr[cp, b*8+i] in fp16.
  - PE: one K=1 ones-matmul adds the routing bias row (its own tiny early
    f32 DMA), then 8 accumulating fp16 matmuls (1 PE cycle/row) -> S [64,16]
    f32 in PSUM.
  - squash: one DVE tensor_tensor_reduce gives nsq = |S/10|^2, ACT Sqrt +
    DVE reciprocal give the coefficient, one dual-scalar DVE mult -> vrow;
    plain SP DMA writes v.
"""

import numpy as np

BS = 512          # full batch
NC = 8            # cores
B = BS // NC      # batches per core
# HEAD batches ride the SP/HWDGE ring in f32 (first byte at ~2.0us, a
# fused 36-fold reduce starts the DVE early); the rest stream through
# Pool/SWDGE cast-DMAs in fp16.  Pool desc-gen costs 994 + 43.5*nb per
# tile against a 204.8ns/batch transfer slot, so bulk tiles stay >= 7
# batches; the taper keeps the final reduce tail short.
HEAD = 2
SUBS = [7, 7, 7, 7, 7, 7, 6, 6, 5, 2, 1]
# Fold methods: tiles 0..6 fold via the 2x-mode fp16 add tree with the
# L2..L6 levels amortized over SPANS (tile-aligned); tiles 7..8 get
# per-tile trees; tile 9 (2 batches) a 2-op fold; tile 10 (1 batch) one
# fused 36-fold reduce — the minimum-latency primitive after the final
# DMA byte.
NSPAN_TILES = 7
SPANS = [14, 14, 14, 6]   # batches, = tile pairs (0-1, 2-3, 4-5) and 6
NTREE_TILES = 9           # tiles 7..8 add per-tile tree levels
CH = 256          # channels
HW = 144          # h*w
Q = 18            # spatial positions per mod-8 bucket
I8 = 8            # in_dim (= p % 8 bucket)
D = 16            # out_dim
NO = 10           # num output caps

assert HEAD + sum(SUBS) == B
assert sum(SPANS) == sum(SUBS[:NSPAN_TILES])


def _build_nc():
    from contextlib import ExitStack

    import concourse.bass as bass
    import concourse.mybir as mybir
    import concourse.tile as tile
    from concourse import bacc

    f32 = mybir.dt.float32
    f16 = mybir.dt.float16
    AF = mybir.ActivationFunctionType

    # Bacc (not plain Bass): its finalize() runs the sync legalization
    # (event semaphores / matmul-wait moves) that splits multi-wait
    # instructions the TRN2 ISA can't encode.
    # 64KB SWDGE scratch = 4096-descriptor ring so the cast-DMA descriptor
    # generation runs well ahead of the transfer slots.
    nc = bacc.Bacc(dynamic_dma_scratch_size=65536)
    x = nc.dram_tensor("x", [B, CH, HW], f32, kind="ExternalInput")
    # fp16 weight matrix wq[p, i*16+d] = W[p//4, d, i] (host pre-packed)
    wq = nc.dram_tensor("wq", [128, I8 * D], f16, kind="ExternalInput")
    # f32 bias row [1, 16] = 144 * sum_s Wb[s, :]
    wb = nc.dram_tensor("wb", [1, D], f32, kind="ExternalInput")
    # one row per batch; the 10 identical caps are replicated host-side
    # during the unshard (they are mathematically equal, see module doc)
    v = nc.dram_tensor("v", [B, D], f32, kind="ExternalOutput")

    with tile.TileContext(nc) as tc, ExitStack() as ctx:
        consts = ctx.enter_context(tc.tile_pool(name="consts", bufs=1))
        xpool = ctx.enter_context(tc.tile_pool(name="xin", bufs=len(SUBS)))
        xhpool = ctx.enter_context(tc.tile_pool(name="xh", bufs=len(SUBS)))
        xrpool = ctx.enter_context(tc.tile_pool(name="xr", bufs=1))
        small = ctx.enter_context(tc.tile_pool(name="small", bufs=1))
        psum = ctx.enter_context(tc.tile_pool(name="psum", bufs=1, space="PSUM"))

        # f32 head on the SP/HWDGE ring: first bytes flow at ~2.0us while
        # the Pool desc-gen for the cast stream is still priming.
        xhead = xpool.tile([128, HEAD * 2 * HW], f32, tag="xhead", bufs=1)
        nc.sync.dma_start(
            xhead[:, :],
            x[0:HEAD].rearrange("b (cp cl) p -> cp b (cl p)", cp=128),
        )

        # x cast-stream: tile [cp, b, (cl p)] fp16; src runs are the
        # 1152B channel-pair lines, dst runs their 576B fp16 images.
        from collections import Counter

        size_counts = Counter(SUBS)
        xts = []
        x_insts = []
        off = HEAD
        for nb in SUBS:
            xt = xpool.tile(
                [128, nb * 2 * HW], f16, tag=f"xt{nb}", bufs=size_counts[nb]
            )
            src = x[off : off + nb].rearrange("b (cp cl) p -> cp b (cl p)", cp=128)
            dst = xt[:, :].rearrange("c (b clp) -> c b clp", clp=2 * HW)
            x_insts.append(nc.gpsimd.dma_start(dst, src))
            xts.append(xt)
            off += nb

        # bias row (tiny, f32) rides the SP/HWDGE ring early.  The fp16
        # weights go through Pool/SWDGE with a no-sync (scheduler-order)
        # dep on the last x tile, so their DMA_ENGINES slot lands BEHIND
        # the final x byte instead of mid-stream: the last reduce then
        # hangs off a byte that is 182ns earlier.
        brow = consts.tile([1, D], f32)
        nc.sync.dma_start(brow[:, :], wb[:, :])
        wpk = consts.tile([128, I8 * D], f16)
        wpk_inst = nc.gpsimd.dma_start(wpk[:, :], wq[:, :])
        from concourse.bass import InstructionNameOrderedSet

        _deps = InstructionNameOrderedSet()
        _deps.add(x_insts[-1].ins.name)
        wpk_inst.ins.add_nosync_dependencies_from(_deps)
        ones = consts.tile([1, B], f32)
        nc.vector.memset(ones[:, :], 1.0)
        # DVE warm-up, then an early ACT Sqrt pins the sqrt_and_others
        # activation table (holds Sqrt, Square and Copy).
        scr = consts.tile([1, 1], f32)
        nc.vector.tensor_copy(scr[:, :], ones[0:1, 0:1])
        scr2 = consts.tile([1, 1], f32)
        nc.scalar.activation(scr2[:, :], scr[:, :], AF.Sqrt)

        # DVE fold.  Bulk: an fp16 binary add-tree, every level in the DVE
        # 2x mode (all operands fp16 with packed innermost i):
        #   L1 (per DMA tile):  t1[c,b,q,i]  = xt[c,b,0,q,i] + xt[c,b,1,q,i]
        #   L2..L6:             fold q 18 -> 9 -> 4(+rem) -> 2 -> 1 (+rem)
        # ~140 DVE cycles/batch vs 288 for a flat reduce.  L2..L6 run over
        # SPANS for the bulk (amortizing instruction inits) and per-tile
        # for tiles 7..8; ops are emitted level-major so no op waits on
        # its immediate predecessor (the ~95ns same-engine RAW stall).
        # Tile 9 (2 batches) folds in 2 ops; tile 10 (1 batch) in ONE
        # fused 36-fold reduce hanging off the final DMA byte.
        xr = xrpool.tile([128, B * I8], f16)
        ntb = sum(SUBS[:NTREE_TILES])  # batches folded via trees
        t1 = xhpool.tile(
            [128, (ntb + SUBS[NTREE_TILES]) * HW], f16, tag="t1", bufs=1
        )
        t2 = xhpool.tile([128, ntb * 9 * I8], f16, tag="t2", bufs=1)
        t3 = xhpool.tile([128, ntb * 4 * I8], f16, tag="t3", bufs=1)
        t4 = xhpool.tile([128, ntb * 2 * I8], f16, tag="t4", bufs=1)
        t5 = xhpool.tile([128, ntb * I8], f16, tag="t5", bufs=1)

        def qv(tile, width, b0, nb, q0, q1):
            """[c, b-range, q0:q1, i] view of a (b, width-q, i) laid tile."""
            v = tile[:, b0 * width * I8 : (b0 + nb) * width * I8]
            return v.rearrange("c (b q i) -> c b q i", q=width, i=I8)[
                :, :, q0:q1
            ]

        def emit_l1(t, b0, eng=None):
            nb = SUBS[t]
            half = xts[t][:, :].rearrange(
                "c (b cl qi) -> c cl b qi", cl=2, qi=HW
            )
            (eng or nc.vector).tensor_tensor(
                out=qv(t1, Q, b0, nb, 0, Q),
                in0=half[:, 0].rearrange("c b (q i) -> c b q i", i=I8),
                in1=half[:, 1].rearrange("c b (q i) -> c b q i", i=I8),
                op=mybir.AluOpType.add,
            )

        LEVELS = [
            (t1, Q, t2, 9, 0, 9, 9, 18),      # 18 -> 9
            (t2, 9, t3, 4, 0, 4, 4, 8),       # 9  -> 4 (+rem q=8)
            (t3, 4, t4, 2, 0, 2, 2, 4),       # 4  -> 2
            (t4, 2, t5, 1, 0, 1, 1, 2),       # 2  -> 1
        ]

        def emit_level(lvl, b0, nb):
            if lvl < 4:
                src, sw, dst, dw, a0, a1, q0, q1 = LEVELS[lvl]
                nc.vector.tensor_tensor(
                    out=qv(dst, dw, b0, nb, 0, dw),
                    in0=qv(src, sw, b0, nb, a0, a1),
                    in1=qv(src, sw, b0, nb, q0, q1),
                    op=mybir.AluOpType.add,
                )
            else:  # L6: xr = t5 + leftover q'=8 of t2
                nc.vector.tensor_tensor(
                    out=qv(xr, 1, HEAD + b0, nb, 0, 1),
                    in0=qv(t5, 1, b0, nb, 0, 1),
                    in1=qv(t2, 9, b0, nb, 8, 9),
                    op=mybir.AluOpType.add,
                )

        def emit_fused(src_ap, xr_b0, nb, clq):
            red_in = src_ap.rearrange(
                "c (b clq i) -> c b i clq", clq=clq, i=I8
            )
            nc.vector.tensor_reduce(
                out=xr[:, xr_b0 * I8 : (xr_b0 + nb) * I8],
                in_=red_in,
                axis=mybir.AxisListType.X,
                op=mybir.AluOpType.add,
            )

        tree_off = []  # batch offset (within tree space) per tile
        o = 0
        for t in range(NTREE_TILES):
            tree_off.append(o)
            o += SUBS[t]

        with nc.allow_low_precision("fp16 fold feeds fp16 matmul"):
            # head: fused f32 36-fold -> fp16 xr rows 0..HEAD
            emit_fused(xhead[:, :], 0, HEAD, 2 * Q)
            # L1 per SWDGE tile (tree tiles)
            for t in range(NTREE_TILES):
                emit_l1(t, tree_off[t])
            # L2..L6 level-major: spans first, then per-tile for 7..8
            for lvl in range(5):
                b0 = 0
                for nb in SPANS:
                    emit_level(lvl, b0, nb)
                    b0 += nb
                for t in range(NSPAN_TILES, NTREE_TILES):
                    emit_level(lvl, tree_off[t], SUBS[t])
            # tile 9 (2 batches): cl-add into t1 scratch + one q-reduce
            t9 = NTREE_TILES
            nb9 = SUBS[t9]
            b9 = HEAD + sum(SUBS[:t9])
            # tile 9's cl-add runs on Pool (idle after desc-gen): it lands
            # before the backlogged DVE would even reach it, removing
            # ~270ns of DVE work from the critical tail.
            emit_l1(t9, ntb, eng=nc.gpsimd)
            xh9 = t1[:, ntb * HW : (ntb + nb9) * HW].rearrange(
                "c (b q i) -> c b i q", q=Q, i=I8
            )
            nc.vector.tensor_reduce(
                out=xr[:, b9 * I8 : (b9 + nb9) * I8],
                in_=xh9,
                axis=mybir.AxisListType.X,
                op=mybir.AluOpType.add,
            )
            # tile 10 (1 batch): single fused 36-fold off the last DMA byte
            t10 = NTREE_TILES + 1
            emit_fused(
                xts[t10][:, :], HEAD + sum(SUBS[:t10]), SUBS[t10], 2 * Q
            )

        # S[b, d] = brow[d] + sum_{p, i} xr[p, b*8+i] * wpk[p, i*16+d]
        # bias via a K=1 ones-matmul (brow arrives ~4us in), then 8
        # accumulating fp16 matmuls at 1 PE cycle/row.
        ps = psum.tile([B, D], f32)
        nc.tensor.matmul(ps[:, :], ones[:, :], brow[:, :], start=True, stop=False)
        xr_v = xr[:, :].rearrange("c (b i) -> c i b", i=I8)
        for i in range(I8):
            nc.tensor.matmul(
                ps[:, :],
                xr_v[:, i, :],
                wpk[:, i * D : (i + 1) * D],
                start=False,
                stop=(i == I8 - 1),
            )

        # squash with m = S/10 folded into the scales:
        #   nsq = |m|^2 = 0.01 * sum_d S^2,  rt = 0.1*sqrt(nsq),
        #   v_row = S * rt / (1 + nsq)
        # (a DVE square would avoid ACT's 187ns accumulator read, but the
        # walrus verifier rejects TensorTensor with two PSUM operands)
        sq = small.tile([B, D], f32)
        nsq = small.tile([B, 1], f32)
        nc.scalar.activation(
            sq[:, :], ps[:, :], AF.Square, scale=0.1, accum_out=nsq[:, :]
        )
        rt = small.tile([B, 1], f32)
        nc.scalar.activation(rt[:, :], nsq[:, :], AF.Sqrt, scale=0.01)
        # den/rec on DVE overlap the ACT Sqrt
        den = small.tile([B, 1], f32)
        nc.vector.tensor_scalar_add(den[:, :], nsq[:, :], 1.0)
        rec = small.tile([B, 1], f32)
        nc.vector.reciprocal(rec[:, :], den[:, :])

        # v_row = (S * rt) * rec in one dual-scalar DVE op
        vrow = small.tile([B, D], f32)
        nc.vector.tensor_scalar(
            vrow[:, :],
            ps[:, :],
            rt[:, :],
            rec[:, :],
            op0=mybir.AluOpType.mult,
            op1=mybir.AluOpType.mult,
        )
        nc.sync.dma_start(v[:, :], vrow[:, :])

    nc.finalize()
    return nc


def _host_inputs(x, W, Wb):
    x = np.ascontiguousarray(np.asarray(x, dtype=np.float32)).reshape(BS, CH, HW)
    W = np.asarray(W, dtype=np.float32)
    Wb = np.asarray(Wb, dtype=np.float32)

    # wq[p, i*16 + d] = W[p//4, d, i]  (channel-pair p covers channels
    # 2p, 2p+1, both in group p//4; the cl-pair sum happens on the DVE)
    wrj = np.empty((I8, 128, D), dtype=np.float32)
    s_of_p = np.arange(128) // 4
    for i in range(I8):
        wrj[i] = W[s_of_p, :, i]
    wq = np.ascontiguousarray(
        wrj.transpose(1, 0, 2).reshape(128, I8 * D).astype(np.float16)
    )

    # brow[d] = 144 * sum_s Wb[s, d]  (the /10 happens in the ACT scale)
    wb = np.ascontiguousarray(HW * Wb.sum(axis=0, dtype=np.float64).astype(np.float32))
    return x, wq, wb.reshape(1, D)


def _run(x, W, Wb, trace=False):
    from concourse.bass_utils import run_bass_kernel_spmd

    xs, wq, wb = _host_inputs(x, W, Wb)
    nc = _build_nc()
    in_maps = [
        {"x": np.ascontiguousarray(xs[k * B : (k + 1) * B]), "wq": wq, "wb": wb}
        for k in range(NC)
    ]
    res = run_bass_kernel_spmd(nc, in_maps, list(range(NC)), trace=trace)
    rows = np.concatenate([res.results[k]["v"] for k in range(NC)], axis=0)
    # unshard: replicate the (identical) caps into the full [BS, NO, D] shape
    out = np.ascontiguousarray(
        np.broadcast_to(rows.reshape(BS, 1, D), (BS, NO, D)), dtype=np.float32
    )
    return out, res


def _numpy_fallback(x, W, Wb, b0):
    """Generic routing on the host — only used if b0 is ever nonzero
    (the spec fills b0 with zeros, which collapses the routing; see top)."""
    x = np.asarray(x, np.float32)
    W = np.asarray(W, np.float32)
    Wb = np.asarray(Wb, np.float32)
    b0 = np.asarray(b0, np.float32)
    u = x.reshape(BS, 32, HW, I8)
    pred = np.einsum("bsni,soi->bsno", u, W) + Wb[None, :, None, :]
    pred = pred.reshape(BS, 32 * HW, D)
    b = np.broadcast_to(b0, (BS,) + b0.shape).copy()
    v = None
    for _ in range(3):
        e = np.exp(b - b.max(axis=1, keepdims=True))
        c = e / e.sum(axis=1, keepdims=True)
        s = np.einsum("boi,bid->bod", c, pred)
        nrm = np.linalg.norm(s, axis=2)
        coeff = (nrm * nrm / (1.0 + nrm * nrm)) / nrm
        v = s * coeff[:, :, None]
        b = b + np.einsum("bid,bod->boi", pred, v)
    return v.astype(np.float32)


def kernel(x, W, Wb, b0=None, **_ignored):
    if b0 is not None and np.any(np.asarray(b0)):
        return _numpy_fallback(x, W, Wb, b0)
    try:
        out, _ = _run(x, W, Wb, trace=False)
    except Exception:
        # one retry: the axon-tunneled device occasionally reports a
        # transient NRT_EXEC_UNIT_UNRECOVERABLE on first touch
        out, _ = _run(x, W, Wb, trace=False)
    return out


def kernel_traced(x, W, Wb, b0=None):
    """Like kernel() but also returns the BassKernelResults (exec_time_ns)."""
    return _run(x, W, Wb, trace=True)


# revision 48
# speedup vs baseline: 1.0160x; 1.0067x over previous
"""Trainium2 Bass kernel for nn_CapLayer_90056874263182.

Math note: the reference initializes routing logits b0 = zeros, so the
softmax over the 10 output caps starts uniform; s, v and delta_b are then
identical across caps, so the logits stay equal across caps through every
routing iteration and the softmax stays uniform forever.  The routing loop
therefore collapses exactly to

    v[b, o, :] = squash((1/10) * sum_i pred[b, i, :])   for every o

and  sum_i pred[b,i,:] = sum_{c,i} xr[b,c,i] * W[c//8,:,i] + 144*sum_s Wb[s,:]
where xr[b,c,i] = sum over the 18 spatial positions p with p%8 == i of
x[b,c,p]  (the row-major reshape maps in_dim to p%8).

Kernel per core (64 batches):
  - x streams in through Pool/SWDGE cast-DMAs (f32 -> fp16).  Source runs
    are 1152B channel-pair lines, destination runs 576B fp16 — still full
    descriptor efficiency, at HALF the SBUF-side bytes.  fp16 keeps 10
    mantissa bits (~5e-4 relative), far inside the 2e-2 gate.
  - DVE folds each tile in two steps: a fully-contiguous fp16 add of the
    channel-pair halves (DVE 2x mode), then a strided q(18)-reduce to
    xr[cp, b*8+i] in fp16.
  - PE: one K=1 ones-matmul adds the routing bias row (its own tiny early
    f32 DMA), then 8 accumulating fp16 matmuls (1 PE cycle/row) -> S [64,16]
    f32 in PSUM.
  - squash: one DVE tensor_tensor_reduce gives nsq = |S/10|^2, ACT Sqrt +
    DVE reciprocal give the coefficient, one dual-scalar DVE mult -> vrow;
    plain SP DMA writes v.
"""

import numpy as np

BS = 512          # full batch
NC = 8            # cores
B = BS // NC      # batches per core
# HEAD batches ride the SP/HWDGE ring in f32 (first byte at ~2.0us, a
# fused 36-fold reduce starts the DVE early); the rest stream through
# Pool/SWDGE cast-DMAs in fp16.  Pool desc-gen costs 994 + 43.5*nb per
# tile against a 204.8ns/batch transfer slot, so bulk tiles stay >= 7
# batches; the taper keeps the final reduce tail short.
HEAD = 2
SUBS = [7, 7, 7, 7, 7, 7, 6, 6, 5, 2, 1]
# Fold methods: tiles 0..6 fold via the 2x-mode fp16 add tree with the
# L2..L6 levels amortized over SPANS (tile-aligned); tiles 7..8 get
# per-tile trees; tile 9 (2 batches) a 2-op fold; tile 10 (1 batch) one
# fused 36-fold reduce — the minimum-latency primitive after the final
# DMA byte.
NSPAN_TILES = 7
SPANS = [14, 14, 14, 6]   # batches, = tile pairs (0-1, 2-3, 4-5) and 6
NTREE_TILES = 9           # tiles 7..8 add per-tile tree levels
CH = 256          # channels
HW = 144          # h*w
Q = 18            # spatial positions per mod-8 bucket
I8 = 8            # in_dim (= p % 8 bucket)
D = 16            # out_dim
NO = 10           # num output caps

assert HEAD + sum(SUBS) == B
assert sum(SPANS) == sum(SUBS[:NSPAN_TILES])


def _build_nc():
    from contextlib import ExitStack

    import concourse.bass as bass
    import concourse.mybir as mybir
    import concourse.tile as tile
    from concourse import bacc

    f32 = mybir.dt.float32
    f16 = mybir.dt.float16
    AF = mybir.ActivationFunctionType

    # Bacc (not plain Bass): its finalize() runs the sync legalization
    # (event semaphores / matmul-wait moves) that splits multi-wait
    # instructions the TRN2 ISA can't encode.
    # 64KB SWDGE scratch = 4096-descriptor ring so the cast-DMA descriptor
    # generation runs well ahead of the transfer slots.
    nc = bacc.Bacc(dynamic_dma_scratch_size=65536)
    x = nc.dram_tensor("x", [B, CH, HW], f32, kind="ExternalInput")
    # fp16 weight matrix wq[p, i*16+d] = W[p//4, d, i] (host pre-packed)
    wq = nc.dram_tensor("wq", [128, I8 * D], f16, kind="ExternalInput")
    # f32 bias row [1, 16] = 144 * sum_s Wb[s, :]
    wb = nc.dram_tensor("wb", [1, D], f32, kind="ExternalInput")
    # one row per batch; the 10 identical caps are replicated host-side
    # during the unshard (they are mathematically equal, see module doc)
    v = nc.dram_tensor("v", [B, D], f32, kind="ExternalOutput")

    with tile.TileContext(nc) as tc, ExitStack() as ctx:
        consts = ctx.enter_context(tc.tile_pool(name="consts", bufs=1))
        xpool = ctx.enter_context(tc.tile_pool(name="xin", bufs=len(SUBS)))
        xhpool = ctx.enter_context(tc.tile_pool(name="xh", bufs=len(SUBS)))
        xrpool = ctx.enter_context(tc.tile_pool(name="xr", bufs=1))
        small = ctx.enter_context(tc.tile_pool(name="small", bufs=1))
        psum = ctx.enter_context(tc.tile_pool(name="psum", bufs=1, space="PSUM"))

        # f32 head on the SP/HWDGE ring: first bytes flow at ~2.0us while
        # the Pool desc-gen for the cast stream is still priming.
        xhead = xpool.tile([128, HEAD * 2 * HW], f32, tag="xhead", bufs=1)
        nc.sync.dma_start(
            xhead[:, :],
            x[0:HEAD].rearrange("b (cp cl) p -> cp b (cl p)", cp=128),
        )

        # x cast-stream: tile [cp, b, (cl p)] fp16; src runs are the
        # 1152B channel-pair lines, dst runs their 576B fp16 images.
        from collections import Counter

        size_counts = Counter(SUBS)
        xts = []
        x_insts = []
        off = HEAD
        for nb in SUBS:
            xt = xpool.tile(
                [128, nb * 2 * HW], f16, tag=f"xt{nb}", bufs=size_counts[nb]
            )
            src = x[off : off + nb].rearrange("b (cp cl) p -> cp b (cl p)", cp=128)
            dst = xt[:, :].rearrange("c (b clp) -> c b clp", clp=2 * HW)
            x_insts.append(nc.gpsimd.dma_start(dst, src))
            xts.append(xt)
            off += nb

        # bias row (tiny, f32) rides the SP/HWDGE ring early.  The fp16
        # weights go through Pool/SWDGE with a no-sync (scheduler-order)
        # dep on the last x tile, so their DMA_ENGINES slot lands BEHIND
        # the final x byte instead of mid-stream: the last reduce then
        # hangs off a byte that is 182ns earlier.
        brow = consts.tile([1, D], f32)
        nc.sync.dma_start(brow[:, :], wb[:, :])
        wpk = consts.tile([128, I8 * D], f16)
        wpk_inst = nc.gpsimd.dma_start(wpk[:, :], wq[:, :])
        from concourse.bass import InstructionNameOrderedSet

        _deps = InstructionNameOrderedSet()
        _deps.add(x_insts[-1].ins.name)
        wpk_inst.ins.add_nosync_dependencies_from(_deps)
        ones = consts.tile([1, B], f32)
        nc.vector.memset(ones[:, :], 1.0)
        # DVE warm-up, then an early ACT Sqrt pins the sqrt_and_others
        # activation table (holds Sqrt, Square and Copy).
        scr = consts.tile([1, 1], f32)
        nc.vector.tensor_copy(scr[:, :], ones[0:1, 0:1])
        scr2 = consts.tile([1, 1], f32)
        nc.scalar.activation(scr2[:, :], scr[:, :], AF.Sqrt)

        # DVE fold.  Bulk: an fp16 binary add-tree, every level in the DVE
        # 2x mode (all operands fp16 with packed innermost i):
        #   L1 (per DMA tile):  t1[c,b,q,i]  = xt[c,b,0,q,i] + xt[c,b,1,q,i]
        #   L2..L6:             fold q 18 -> 9 -> 4(+rem) -> 2 -> 1 (+rem)
        # ~140 DVE cycles/batch vs 288 for a flat reduce.  L2..L6 run over
        # SPANS for the bulk (amortizing instruction inits) and per-tile
        # for tiles 7..8; ops are emitted level-major so no op waits on
        # its immediate predecessor (the ~95ns same-engine RAW stall).
        # Tile 9 (2 batches) folds in 2 ops; tile 10 (1 batch) in ONE
        # fused 36-fold reduce hanging off the final DMA byte.
        xr = xrpool.tile([128, B * I8], f16)
        ntb = sum(SUBS[:NTREE_TILES])  # batches folded via trees
        t1 = xhpool.tile(
            [128, (ntb + SUBS[NTREE_TILES] + SUBS[NTREE_TILES + 1]) * HW],
            f16,
            tag="t1",
            bufs=1,
        )
        t2 = xhpool.tile([128, ntb * 9 * I8], f16, tag="t2", bufs=1)
        t3 = xhpool.tile([128, ntb * 4 * I8], f16, tag="t3", bufs=1)
        t4 = xhpool.tile([128, ntb * 2 * I8], f16, tag="t4", bufs=1)
        t5 = xhpool.tile([128, ntb * I8], f16, tag="t5", bufs=1)

        def qv(tile, width, b0, nb, q0, q1):
            """[c, b-range, q0:q1, i] view of a (b, width-q, i) laid tile."""
            v = tile[:, b0 * width * I8 : (b0 + nb) * width * I8]
            return v.rearrange("c (b q i) -> c b q i", q=width, i=I8)[
                :, :, q0:q1
            ]

        def emit_l1(t, b0, eng=None):
            nb = SUBS[t]
            half = xts[t][:, :].rearrange(
                "c (b cl qi) -> c cl b qi", cl=2, qi=HW
            )
            (eng or nc.vector).tensor_tensor(
                out=qv(t1, Q, b0, nb, 0, Q),
                in0=half[:, 0].rearrange("c b (q i) -> c b q i", i=I8),
                in1=half[:, 1].rearrange("c b (q i) -> c b q i", i=I8),
                op=mybir.AluOpType.add,
            )

        LEVELS = [
            (t1, Q, t2, 9, 0, 9, 9, 18),      # 18 -> 9
            (t2, 9, t3, 4, 0, 4, 4, 8),       # 9  -> 4 (+rem q=8)
            (t3, 4, t4, 2, 0, 2, 2, 4),       # 4  -> 2
            (t4, 2, t5, 1, 0, 1, 1, 2),       # 2  -> 1
        ]

        def emit_level(lvl, b0, nb):
            if lvl < 4:
                src, sw, dst, dw, a0, a1, q0, q1 = LEVELS[lvl]
                nc.vector.tensor_tensor(
                    out=qv(dst, dw, b0, nb, 0, dw),
                    in0=qv(src, sw, b0, nb, a0, a1),
                    in1=qv(src, sw, b0, nb, q0, q1),
                    op=mybir.AluOpType.add,
                )
            else:  # L6: xr = t5 + leftover q'=8 of t2
                nc.vector.tensor_tensor(
                    out=qv(xr, 1, HEAD + b0, nb, 0, 1),
                    in0=qv(t5, 1, b0, nb, 0, 1),
                    in1=qv(t2, 9, b0, nb, 8, 9),
                    op=mybir.AluOpType.add,
                )

        def emit_fused(src_ap, xr_b0, nb, clq):
            red_in = src_ap.rearrange(
                "c (b clq i) -> c b i clq", clq=clq, i=I8
            )
            nc.vector.tensor_reduce(
                out=xr[:, xr_b0 * I8 : (xr_b0 + nb) * I8],
                in_=red_in,
                axis=mybir.AxisListType.X,
                op=mybir.AluOpType.add,
            )

        tree_off = []  # batch offset (within tree space) per tile
        o = 0
        for t in range(NTREE_TILES):
            tree_off.append(o)
            o += SUBS[t]

        with nc.allow_low_precision("fp16 fold feeds fp16 matmul"):
            # head: fused f32 36-fold -> fp16 xr rows 0..HEAD
            emit_fused(xhead[:, :], 0, HEAD, 2 * Q)
            # L1 per SWDGE tile (tree tiles)
            for t in range(NTREE_TILES):
                emit_l1(t, tree_off[t])
            # L2..L6 level-major: spans first, then per-tile for 7..8
            for lvl in range(5):
                b0 = 0
                for nb in SPANS:
                    emit_level(lvl, b0, nb)
                    b0 += nb
                for t in range(NSPAN_TILES, NTREE_TILES):
                    emit_level(lvl, tree_off[t], SUBS[t])
            # tile 9 (2 batches): cl-add into t1 scratch + one q-reduce
            t9 = NTREE_TILES
            nb9 = SUBS[t9]
            b9 = HEAD + sum(SUBS[:t9])
            # tile 9's cl-add runs on Pool (idle after desc-gen): it lands
            # before the backlogged DVE would even reach it, removing
            # ~270ns of DVE work from the critical tail.
            emit_l1(t9, ntb, eng=nc.gpsimd)
            xh9 = t1[:, ntb * HW : (ntb + nb9) * HW].rearrange(
                "c (b q i) -> c b i q", q=Q, i=I8
            )
            nc.vector.tensor_reduce(
                out=xr[:, b9 * I8 : (b9 + nb9) * I8],
                in_=xh9,
                axis=mybir.AxisListType.X,
                op=mybir.AluOpType.add,
            )
            # tile 10 (1 batch): same Pool-assist — cl-add on Pool, then a
            # 1-batch q-reduce (210ns) instead of a fused 36-fold (360ns)
            # hanging off the last DMA byte.
            t10 = NTREE_TILES + 1
            nb10 = SUBS[t10]
            b10 = HEAD + sum(SUBS[:t10])
            emit_l1(t10, ntb + nb9, eng=nc.gpsimd)
            xh10 = t1[
                :, (ntb + nb9) * HW : (ntb + nb9 + nb10) * HW
            ].rearrange("c (b q i) -> c b i q", q=Q, i=I8)
            nc.vector.tensor_reduce(
                out=xr[:, b10 * I8 : (b10 + nb10) * I8],
                in_=xh10,
                axis=mybir.AxisListType.X,
                op=mybir.AluOpType.add,
            )

        # S[b, d] = brow[d] + sum_{p, i} xr[p, b*8+i] * wpk[p, i*16+d]
        # bias via a K=1 ones-matmul (brow arrives ~4us in), then 8
        # accumulating fp16 matmuls at 1 PE cycle/row.
        ps = psum.tile([B, D], f32)
        nc.tensor.matmul(ps[:, :], ones[:, :], brow[:, :], start=True, stop=False)
        xr_v = xr[:, :].rearrange("c (b i) -> c i b", i=I8)
        for i in range(I8):
            nc.tensor.matmul(
                ps[:, :],
                xr_v[:, i, :],
                wpk[:, i * D : (i + 1) * D],
                start=False,
                stop=(i == I8 - 1),
            )

        # squash with m = S/10 folded into the scales:
        #   nsq = |m|^2 = 0.01 * sum_d S^2,  rt = 0.1*sqrt(nsq),
        #   v_row = S * rt / (1 + nsq)
        # (a DVE square would avoid ACT's 187ns accumulator read, but the
        # walrus verifier rejects TensorTensor with two PSUM operands)
        sq = small.tile([B, D], f32)
        nsq = small.tile([B, 1], f32)
        nc.scalar.activation(
            sq[:, :], ps[:, :], AF.Square, scale=0.1, accum_out=nsq[:, :]
        )
        rt = small.tile([B, 1], f32)
        nc.scalar.activation(rt[:, :], nsq[:, :], AF.Sqrt, scale=0.01)
        # den/rec on DVE overlap the ACT Sqrt
        den = small.tile([B, 1], f32)
        nc.vector.tensor_scalar_add(den[:, :], nsq[:, :], 1.0)
        rec = small.tile([B, 1], f32)
        nc.vector.reciprocal(rec[:, :], den[:, :])

        # v_row = (S * rt) * rec in one dual-scalar DVE op
        vrow = small.tile([B, D], f32)
        nc.vector.tensor_scalar(
            vrow[:, :],
            ps[:, :],
            rt[:, :],
            rec[:, :],
            op0=mybir.AluOpType.mult,
            op1=mybir.AluOpType.mult,
        )
        nc.sync.dma_start(v[:, :], vrow[:, :])

    nc.finalize()
    return nc


def _host_inputs(x, W, Wb):
    x = np.ascontiguousarray(np.asarray(x, dtype=np.float32)).reshape(BS, CH, HW)
    W = np.asarray(W, dtype=np.float32)
    Wb = np.asarray(Wb, dtype=np.float32)

    # wq[p, i*16 + d] = W[p//4, d, i]  (channel-pair p covers channels
    # 2p, 2p+1, both in group p//4; the cl-pair sum happens on the DVE)
    wrj = np.empty((I8, 128, D), dtype=np.float32)
    s_of_p = np.arange(128) // 4
    for i in range(I8):
        wrj[i] = W[s_of_p, :, i]
    wq = np.ascontiguousarray(
        wrj.transpose(1, 0, 2).reshape(128, I8 * D).astype(np.float16)
    )

    # brow[d] = 144 * sum_s Wb[s, d]  (the /10 happens in the ACT scale)
    wb = np.ascontiguousarray(HW * Wb.sum(axis=0, dtype=np.float64).astype(np.float32))
    return x, wq, wb.reshape(1, D)


def _run(x, W, Wb, trace=False):
    from concourse.bass_utils import run_bass_kernel_spmd

    xs, wq, wb = _host_inputs(x, W, Wb)
    nc = _build_nc()
    in_maps = [
        {"x": np.ascontiguousarray(xs[k * B : (k + 1) * B]), "wq": wq, "wb": wb}
        for k in range(NC)
    ]
    res = run_bass_kernel_spmd(nc, in_maps, list(range(NC)), trace=trace)
    rows = np.concatenate([res.results[k]["v"] for k in range(NC)], axis=0)
    # unshard: replicate the (identical) caps into the full [BS, NO, D] shape
    out = np.ascontiguousarray(
        np.broadcast_to(rows.reshape(BS, 1, D), (BS, NO, D)), dtype=np.float32
    )
    return out, res


def _numpy_fallback(x, W, Wb, b0):
    """Generic routing on the host — only used if b0 is ever nonzero
    (the spec fills b0 with zeros, which collapses the routing; see top)."""
    x = np.asarray(x, np.float32)
    W = np.asarray(W, np.float32)
    Wb = np.asarray(Wb, np.float32)
    b0 = np.asarray(b0, np.float32)
    u = x.reshape(BS, 32, HW, I8)
    pred = np.einsum("bsni,soi->bsno", u, W) + Wb[None, :, None, :]
    pred = pred.reshape(BS, 32 * HW, D)
    b = np.broadcast_to(b0, (BS,) + b0.shape).copy()
    v = None
    for _ in range(3):
        e = np.exp(b - b.max(axis=1, keepdims=True))
        c = e / e.sum(axis=1, keepdims=True)
        s = np.einsum("boi,bid->bod", c, pred)
        nrm = np.linalg.norm(s, axis=2)
        coeff = (nrm * nrm / (1.0 + nrm * nrm)) / nrm
        v = s * coeff[:, :, None]
        b = b + np.einsum("bid,bod->boi", pred, v)
    return v.astype(np.float32)


def kernel(x, W, Wb, b0=None, **_ignored):
    if b0 is not None and np.any(np.asarray(b0)):
        return _numpy_fallback(x, W, Wb, b0)
    try:
        out, _ = _run(x, W, Wb, trace=False)
    except Exception:
        # one retry: the axon-tunneled device occasionally reports a
        # transient NRT_EXEC_UNIT_UNRECOVERABLE on first touch
        out, _ = _run(x, W, Wb, trace=False)
    return out


def kernel_traced(x, W, Wb, b0=None):
    """Like kernel() but also returns the BassKernelResults (exec_time_ns)."""
    return _run(x, W, Wb, trace=True)
